# revision 1
# baseline (speedup 1.0000x reference)
"""Trainium2 Bass kernel for Gaussian-KDE logsumexp (nn_GaussianKernel).

out[n] = logsumexp_m( -0.5*||(y_n - x_m)/bw||^2 - Z ),  Z = D/2*log(2pi) + D*log(bw) + log(M)

Factorization used on-device (per query row n, data col m):
    A[n,m] = (y_n . x_m)/bw^2  -  ||x_m||^2/(2 bw^2)
    out[n] = max_m A[n,m] + log(sum_m exp(A[n,m] - max)) - ||y_n||^2/(2 bw^2) - Z

Sharding: data-parallel over the 2048 query rows -> 8 cores x 256 rows,
each core holds the full x dataset (matmul K=D=128 on partitions).

Per core: A is built in PSUM as two accumulating matmul passes per
512-col bank (rank-128 bias pass with a constant matrix computing
-||x_m||^2/(2bw^2) replicated over partitions, plus the main y.x pass),
using float32r (full-rate fp32 PE mode). The logsumexp is one coarse DVE
max (negated) + one coarse ACT Exp with fused row-sum accumulation per
128-row tile, then Ln + per-partition affine combine.
"""

import sys
from math import log, pi

import numpy as np

sys.path.insert(0, "/opt/trn_rl_repo")

import concourse.bacc as bacc
import concourse.bass as bass
import concourse.mybir as mybir
import concourse.tile as tile
from concourse.bass_utils import run_bass_kernel_spmd

BW = 0.1
N_QUERY = 2048
N_DATA = 2048
DIM = 128
N_CORES = 8
SHARD = N_QUERY // N_CORES  # 256 query rows per core

NEG_HALF_INV_BW2 = -0.5 / (BW * BW)  # -50.0
Z_CONST = 0.5 * DIM * log(2.0 * pi) + DIM * log(BW) + log(float(N_DATA))

NT = 512  # one PSUM bank of fp32
N_TILES = N_DATA // NT  # 4
M_TILES = SHARD // 128  # 2

_CACHE = {}


def _build_nc():
    dt = mybir.dt.float32
    f32r = mybir.dt.float32r
    fx = mybir.ActivationFunctionType
    nc = bacc.Bacc("TRN2", target_bir_lowering=False, debug=False)

    # Inputs (pre-laid-out on host): yt = (y_shard/bw^2).T, xt = x.T, ynat = y_shard
    yt = nc.dram_tensor("yt", [DIM, SHARD], f32r, kind="ExternalInput")
    xt = nc.dram_tensor("xt", [DIM, N_DATA], f32r, kind="ExternalInput")
    ynat = nc.dram_tensor("ynat", [SHARD, DIM], dt, kind="ExternalInput")
    cmat_d = nc.dram_tensor("cmat", [DIM, 128], f32r, kind="ExternalInput")
    out = nc.dram_tensor("out", [128, M_TILES], dt, kind="ExternalOutput")

    with tile.TileContext(nc) as tc:
        with (
            tc.tile_pool(name="io", bufs=1) as io,
            tc.tile_pool(name="psum", bufs=2, space=bass.MemorySpace.PSUM) as psum,
            tc.tile_pool(name="work", bufs=2) as work,
            tc.tile_pool(name="small", bufs=2) as small,
        ):
            cmat = io.tile([DIM, 128], f32r, tag="cmat")
            nc.sync.dma_start(cmat[:], cmat_d[:])

            # ---- loads; order puts the first matmul's deps first ----
            xt_sb = io.tile([DIM, N_DATA], f32r, tag="xt")
            yt_sb = io.tile([DIM, SHARD], f32r, tag="yt")
            xsq_sb = io.tile([DIM, N_DATA], f32r, tag="xsq")
            ynat_tiles = []
            for mt in range(M_TILES):
                t_ = io.tile([128, DIM], dt, tag=f"yn{mt}", name=f"ynat_sb{mt}")
                ynat_tiles.append(t_)

            def load_chunk(t):
                nc.sync.dma_start(xt_sb[:, t * NT:(t + 1) * NT],
                                  xt[:, t * NT:(t + 1) * NT])
                xt_f32 = xt_sb[:, t * NT:(t + 1) * NT].bitcast(dt)
                nc.gpsimd.tensor_tensor(xsq_sb[:, t * NT:(t + 1) * NT],
                                        xt_f32, xt_f32,
                                        op=mybir.AluOpType.mult)

            load_chunk(0)
            nc.sync.dma_start(yt_sb[:], yt[:])
            for t in range(1, N_TILES):
                load_chunk(t)
            for mt in range(M_TILES):
                nc.sync.dma_start(ynat_tiles[mt][:], ynat[mt * 128:(mt + 1) * 128, :])

            xtr = xt_sb
            xsqr = xsq_sb
            ytr = yt_sb
            cmatr = cmat

            nmaxs, tots, yn2s = [], [], []
            for mt in range(M_TILES):
                # ---- PE: A = yt.T @ xt + cmat.T @ xsq per 512-col bank ----
                A = psum.tile([128, N_DATA], dt, tag="A", name=f"A{mt}")
                for t in range(N_TILES):
                    nc.tensor.matmul(A[:, t * NT:(t + 1) * NT],
                                     ytr[:, mt * 128:(mt + 1) * 128],
                                     xtr[:, t * NT:(t + 1) * NT],
                                     start=True, stop=False)
                for t in range(N_TILES):
                    nc.tensor.matmul(A[:, t * NT:(t + 1) * NT],
                                     cmatr[:],
                                     xsqr[:, t * NT:(t + 1) * NT],
                                     start=False, stop=True)

                # ---- DVE: -rowmax over all 2048 cols in one op ----
                nmax = small.tile([128, 1], dt, tag="nmax", name=f"nmax{mt}")
                nc.vector.tensor_reduce(nmax[:], A[:],
                                        axis=mybir.AxisListType.X,
                                        op=mybir.AluOpType.max, negate=True)

                # ---- ACT: exp(A - max) + fused full-row sum ----
                esc = work.tile([128, N_DATA], dt, tag="esc", name=f"esc{mt}")
                tot = small.tile([128, 1], dt, tag="tot", name=f"tot{mt}")
                nc.scalar.activation(esc[:], A[:], fx.Exp,
                                     bias=nmax[:], scale=1.0,
                                     accum_out=tot[:])

                # ---- ||y_n||^2 ----
                ysq = small.tile([128, DIM], dt, tag="ysq", name=f"ysq{mt}")
                nc.gpsimd.tensor_tensor(ysq[:], ynat_tiles[mt][:], ynat_tiles[mt][:],
                                        op=mybir.AluOpType.mult)
                yn2 = small.tile([128, 1], dt, tag="yn2", name=f"yn2{mt}")
                nc.vector.tensor_reduce(yn2[:], ysq[:],
                                        axis=mybir.AxisListType.X,
                                        op=mybir.AluOpType.add)
                nmaxs.append(nmax)
                tots.append(tot)
                yn2s.append(yn2)

            # ---- Ln for both tiles together (one ACT table switch) ----
            osb = small.tile([128, M_TILES], dt, tag="osb")
            for mt in range(M_TILES):
                lnt = small.tile([128, 1], dt, tag="lnt", name=f"lnt{mt}")
                nc.scalar.activation(lnt[:], tots[mt][:], fx.Ln)
                t1 = small.tile([128, 1], dt, tag="t1", name=f"t1_{mt}")
                nc.vector.tensor_sub(t1[:], lnt[:], nmaxs[mt][:])
                t2 = small.tile([128, 1], dt, tag="t2", name=f"t2_{mt}")
                nc.vector.tensor_scalar(t2[:], yn2s[mt][:], NEG_HALF_INV_BW2,
                                        -Z_CONST,
                                        op0=mybir.AluOpType.mult,
                                        op1=mybir.AluOpType.add)
                nc.vector.tensor_add(osb[:, mt:mt + 1], t1[:], t2[:])

            nc.sync.dma_start(out[:], osb[:])

    nc.compile()
    return nc


def kernel(y, x):
    y = np.asarray(y, dtype=np.float32)
    x = np.asarray(x, dtype=np.float32)
    assert y.shape == (N_QUERY, DIM) and x.shape == (N_DATA, DIM)

    if "nc" not in _CACHE:
        _CACHE["nc"] = _build_nc()
    nc = _CACHE["nc"]

    xt = np.ascontiguousarray(x.T)
    in_maps = []
    for i in range(N_CORES):
        ysh = y[i * SHARD:(i + 1) * SHARD]
        in_maps.append({
            "yt": np.ascontiguousarray(ysh.T) * np.float32(1.0 / (BW * BW)),
            "ynat": np.ascontiguousarray(ysh),
            "cmat": np.full((DIM, 128), NEG_HALF_INV_BW2, dtype=np.float32),
            "xt": xt,
        })

    res = run_bass_kernel_spmd(nc, in_maps, core_ids=list(range(N_CORES)))
    # out[p, mt] holds query row mt*128+p of the core's shard
    return np.concatenate(
        [r["out"].T.reshape(-1) for r in res.results]).astype(np.float32)

